# revision 4
# baseline (speedup 1.0000x reference)
"""Trainium2 Bass kernel for batched masked attention.

Problem: q,k,v [16, 2048, 512] fp32, mask [16, 2048, 2048] bool (True = masked
out).  Returns (out, attn) like the reference:
    attn = softmax((q @ k^T)/sqrt(512) masked with -inf)
    out  = attn @ v

Sharding: batch dim 16 split across 8 NeuronCores, 2 batches per core.

Per-core design (all matmuls bf16, accumulation fp32):
  - Q,K,V loaded with SWDGE cast fp32->bf16; Q,K transposed on PE (d on
    partitions) for the QK^T contraction.
  - S = Q@K^T built per 128-row q-tile in PSUM [128,512] chunks (4 d-step
    matmuls), plus one extra matmul lhsT=(-9984*I128), rhs=mask_bf16 that adds
    -9984 to masked logits in-place in PSUM (exp then underflows to 0).
  - ACT exp reads PSUM, applies the 1/sqrt(512) softmax scale via the free
    affine `scale`, writes bf16 P and emits per-partition row-sum partials via
    accum_out (fp32) -- softmax denominators for free.
  - DVE: sum partials, reciprocal, normalize P (bf16 4x mode); normalized P is
    DMA'd to HBM with SWDGE cast bf16->fp32 as the attn output.
  - P (unnormalized) is PE-transposed per 128x128 tile into PSUM, copied to
    SBUF, used as lhsT for the P@V matmul; PV output rows are scaled by the
    reciprocal row-sum on ACT (copy w/ scale AP) and stored.
"""

import math

import numpy as np

B, S, D = 16, 2048, 512
N_CORES = 8
BPC = B // N_CORES  # batches per core
SOFTMAX_SCALE = 1.0 / math.sqrt(512.0)
MASK_NEG = -9984.0  # exactly representable in bf16; -9984/sqrt(512) ~ -441

P = 128  # partitions


def build_nc(bpc=BPC, s=S, d=D):
    import concourse.mybir as mybir
    import concourse.tile as tile
    from concourse import bacc

    fp32 = mybir.dt.float32
    bf16 = mybir.dt.bfloat16
    u8 = mybir.dt.uint8

    st = s // P      # number of 128-row s-tiles (16)
    dt_n = d // P    # number of 128-row d-tiles (4)
    kc_n = s // 512  # number of 512-wide k chunks (4)

    nc = bacc.Bacc("TRN2", target_bir_lowering=False, debug=False)

    q_dram = nc.dram_tensor("q", [bpc, s, d], fp32, kind="ExternalInput").ap()
    k_dram = nc.dram_tensor("k", [bpc, s, d], fp32, kind="ExternalInput").ap()
    v_dram = nc.dram_tensor("v", [bpc, s, d], fp32, kind="ExternalInput").ap()
    m_dram = nc.dram_tensor("mask", [bpc, s, s], u8, kind="ExternalInput").ap()
    out_dram = nc.dram_tensor("out", [bpc, s, d], fp32, kind="ExternalOutput").ap()
    attn_dram = nc.dram_tensor("attn", [bpc, s, s], fp32, kind="ExternalOutput").ap()

    with tile.TileContext(nc) as tc:
        with (
            tc.tile_pool(name="singles", bufs=1) as singles,
            tc.tile_pool(name="big", bufs=2) as big,
            tc.tile_pool(name="work", bufs=3) as work,
            tc.tile_pool(name="pt", bufs=2) as ptpool,
            tc.tile_pool(name="small", bufs=4) as small,
            tc.tile_pool(name="ps_s", bufs=4, space="PSUM") as ps_s,
            tc.tile_pool(name="ps_t", bufs=2, space="PSUM") as ps_t,
            tc.tile_pool(name="ps_o", bufs=2, space="PSUM") as ps_o,
        ):
            # constants: identity (transpose helper) and -9984*I (mask add)
            ident = singles.tile([P, P], bf16)
            nc.gpsimd.memset(ident, 0.0)
            nc.gpsimd.affine_select(
                out=ident, in_=ident,
                compare_op=mybir.AluOpType.not_equal, fill=1.0,
                base=0, pattern=[[-1, P]], channel_multiplier=1,
            )
            neg_ident = singles.tile([P, P], bf16)
            nc.gpsimd.memset(neg_ident, 0.0)
            nc.gpsimd.affine_select(
                out=neg_ident, in_=neg_ident,
                compare_op=mybir.AluOpType.not_equal, fill=MASK_NEG,
                base=0, pattern=[[-1, P]], channel_multiplier=1,
            )

            def load_transposed(src_dram_b, name):
                """Load [s, d] fp32 DRAM -> SBUF bf16 [P, dt_n, s] with d on
                partitions (via natural load + PE transpose)."""
                nat = big.tile([P, st, d], bf16, name="nat", tag="nat")
                nc.gpsimd.dma_start(
                    out=nat, in_=src_dram_b.rearrange("(t p) d -> p t d", p=P)
                )
                tsb = big.tile([P, dt_n, s], bf16, name=name)
                for dti in range(dt_n):
                    for sti in range(st):
                        if sti % 4 == 0:
                            ptp = ps_t.tile([P, 4, P], bf16, name="ptp", tag="ptp")
                        nc.tensor.transpose(
                            ptp[:, sti % 4, :],
                            nat[:, sti, dti * P:(dti + 1) * P],
                            ident,
                        )
                        if sti % 4 == 3:
                            nc.vector.tensor_copy(
                                out=tsb[:, dti, (sti - 3) * P:(sti + 1) * P],
                                in_=ptp.rearrange("p a b -> p (a b)"),
                            )
                return tsb

            for b in range(bpc):
                qt_sb = load_transposed(q_dram[b], "qt_sb")
                kt_sb = load_transposed(k_dram[b], "kt_sb")
                v_sb = big.tile([P, st, d], bf16, name="v_sb")
                nc.gpsimd.dma_start(
                    out=v_sb, in_=v_dram[b].rearrange("(t p) d -> p t d", p=P)
                )

                for qt in range(st):
                    qsl = slice(qt * P, (qt + 1) * P)
                    mask_bf = work.tile([P, s], bf16, name="mask_bf")
                    nc.gpsimd.dma_start(out=mask_bf, in_=m_dram[b, qsl, :])

                    p_bf = work.tile([P, s], bf16, name="p_bf")
                    rs_parts = small.tile([P, kc_n], fp32, name="rs_parts")
                    for kc in range(kc_n):
                        ksl = slice(kc * 512, (kc + 1) * 512)
                        ps = ps_s.tile([P, 512], fp32, name="ps")
                        for dti in range(dt_n):
                            nc.tensor.matmul(
                                ps,
                                lhsT=qt_sb[:, dti, qsl],
                                rhs=kt_sb[:, dti, ksl],
                                start=(dti == 0),
                                stop=False,
                            )
                        nc.tensor.matmul(
                            ps, lhsT=neg_ident, rhs=mask_bf[:, ksl],
                            start=False, stop=True,
                        )
                        nc.scalar.activation(
                            out=p_bf[:, ksl], in_=ps,
                            func=mybir.ActivationFunctionType.Exp,
                            scale=SOFTMAX_SCALE,
                            accum_out=rs_parts[:, kc:kc + 1],
                        )

                    rinv = small.tile([P, 1], fp32, name="rinv")
                    nc.vector.reduce_sum(
                        out=rinv, in_=rs_parts, axis=mybir.AxisListType.X
                    )
                    nc.vector.reciprocal(out=rinv, in_=rinv)

                    p_norm = work.tile([P, s], bf16, name="p_norm")
                    nc.vector.tensor_scalar_mul(p_norm, p_bf, rinv)
                    nc.gpsimd.dma_start(out=attn_dram[b, qsl, :], in_=p_norm)

                    # transpose P for the PV contraction (k on partitions)
                    pt_sb = ptpool.tile([P, st, P], bf16, name="pt_sb")
                    for kt in range(st):
                        if kt % 4 == 0:
                            ptp = ps_t.tile([P, 4, P], bf16, name="ptp", tag="ptp")
                        nc.tensor.transpose(
                            ptp[:, kt % 4, :], p_bf[:, kt * P:(kt + 1) * P], ident
                        )
                        if kt % 4 == 3:
                            nc.vector.tensor_copy(
                                out=pt_sb[:, kt - 3:kt + 1, :].rearrange(
                                    "p a b -> p (a b)"
                                ),
                                in_=ptp.rearrange("p a b -> p (a b)"),
                            )

                    ps_out = ps_o.tile([P, d], fp32, name="ps_out")
                    for kt in range(st):
                        nc.tensor.matmul(
                            ps_out,
                            lhsT=pt_sb[:, kt, :],
                            rhs=v_sb[:, kt, :],
                            start=(kt == 0),
                            stop=(kt == st - 1),
                        )
                    out_sb = work.tile([P, d], fp32, name="out_sb")
                    nc.scalar.mul(out_sb, ps_out, rinv)
                    nc.sync.dma_start(out=out_dram[b, qsl, :], in_=out_sb)

    nc.compile()
    return nc


_CACHE = {}


def _get_nc():
    if "nc" not in _CACHE:
        _CACHE["nc"] = build_nc()
    return _CACHE["nc"]


def _get_exec():
    """Compile (once) the 8-core shard_map executable wrapping the Bass NEFF.

    Mirrors concourse.bass2jax.run_bass_via_pjrt's multi-core path, but keeps
    the jitted callable so repeat executions don't re-trace/re-compile and so
    inputs can be pre-placed on device for exec-only timing.
    """
    if "exec" in _CACHE:
        return _CACHE["exec"]

    import jax
    from jax.sharding import Mesh, PartitionSpec
    import concourse.mybir as mybir
    from concourse import bass2jax

    try:
        from jax.experimental.shard_map import shard_map
    except ImportError:  # newer jax
        from jax.sharding import shard_map  # type: ignore

    bass2jax.install_neuronx_cc_hook()
    nc = _get_nc()

    partition_name = (
        nc.partition_id_tensor.name if nc.partition_id_tensor else None
    )
    in_names, out_names, out_avals = [], [], []
    for alloc in nc.m.functions[0].allocations:
        if not isinstance(alloc, mybir.MemoryLocationSet):
            continue
        name = alloc.memorylocations[0].name
        if alloc.kind == "ExternalInput":
            if name != partition_name:
                in_names.append(name)
        elif alloc.kind == "ExternalOutput":
            out_names.append(name)
            out_avals.append(
                jax.core.ShapedArray(
                    tuple(alloc.tensor_shape), mybir.dt.np(alloc.dtype)
                )
            )
    n_params = len(in_names)
    n_outs = len(out_avals)
    all_in_names = list(in_names) + list(out_names)
    if partition_name is not None:
        all_in_names.append(partition_name)
    donate = tuple(range(n_params, n_params + n_outs))

    def _body(*args):
        operands = list(args)
        if partition_name is not None:
            operands.append(bass2jax.partition_id_tensor())
        outs = bass2jax._bass_exec_p.bind(
            *operands,
            out_avals=tuple(out_avals),
            in_names=tuple(all_in_names),
            out_names=tuple(out_names),
            lowering_input_output_aliases=(),
            sim_require_finite=True,
            sim_require_nnan=True,
            nc=nc,
        )
        return tuple(outs)

    devices = jax.devices()[:N_CORES]
    mesh = Mesh(np.asarray(devices), ("core",))
    in_specs = (PartitionSpec("core"),) * (n_params + n_outs)
    out_specs = (PartitionSpec("core"),) * n_outs
    sharded = jax.jit(
        shard_map(
            _body, mesh=mesh, in_specs=in_specs, out_specs=out_specs,
            check_rep=False,
        ),
        donate_argnums=donate,
        keep_unused=True,
    )
    _CACHE["exec"] = (sharded, mesh, in_names, out_names, out_avals)
    return _CACHE["exec"]


def _prep_inputs(q, k, v, mask):
    q = np.ascontiguousarray(np.asarray(q, dtype=np.float32))
    k = np.ascontiguousarray(np.asarray(k, dtype=np.float32))
    v = np.ascontiguousarray(np.asarray(v, dtype=np.float32))
    mask_u8 = np.ascontiguousarray(np.asarray(mask)).astype(np.uint8)
    return {"q": q, "k": k, "v": v, "mask": mask_u8}


def _device_args(arrs):
    """Place global inputs + fresh zero output buffers on the mesh."""
    import jax
    from jax.sharding import NamedSharding, PartitionSpec

    sharded, mesh, in_names, out_names, out_avals = _get_exec()
    sh = NamedSharding(mesh, PartitionSpec("core"))
    args = [jax.device_put(arrs[n], sh) for n in in_names]
    for av in out_avals:
        z = np.zeros((N_CORES * av.shape[0], *av.shape[1:]), av.dtype)
        args.append(jax.device_put(z, sh))
    return args


def run(q, k, v, mask):
    import jax

    arrs = _prep_inputs(q, k, v, mask)
    sharded, mesh, in_names, out_names, out_avals = _get_exec()
    args = _device_args(arrs)
    outs = jax.block_until_ready(sharded(*args))
    by_name = {n: np.asarray(outs[i]) for i, n in enumerate(out_names)}
    return by_name["out"], by_name["attn"]


def timed_run(q, k, v, mask, iters=3):
    """Returns (outputs, per-iteration wall seconds for exec-only calls)."""
    import time

    import jax

    arrs = _prep_inputs(q, k, v, mask)
    sharded, mesh, in_names, out_names, out_avals = _get_exec()

    args = _device_args(arrs)
    outs = jax.block_until_ready(sharded(*args))  # warm-up / compile
    times = []
    for _ in range(iters):
        args = _device_args(arrs)
        jax.block_until_ready(args)
        t0 = time.perf_counter()
        o = jax.block_until_ready(sharded(*args))
        times.append(time.perf_counter() - t0)
        del o
    by_name = {n: np.asarray(outs[i]) for i, n in enumerate(out_names)}
    return (by_name["out"], by_name["attn"]), times


def kernel(q, k, v, mask):
    out, attn = run(q, k, v, mask)
    return out, attn


# revision 14
# speedup vs baseline: 91.8497x; 91.8497x over previous
"""Trainium2 Bass kernel for batched masked attention.

Problem: q,k,v [16, 2048, 512] fp32, mask [16, 2048, 2048] bool (True = masked
out).  Returns (out, attn) like the reference:
    attn = softmax((q @ k^T)/sqrt(512) masked with -inf)
    out  = attn @ v

Sharding: batch dim 16 split across 8 NeuronCores, 2 batches per core.

Per-core design (all matmuls bf16, accumulation fp32):
  - Q,K,V loaded with SWDGE cast fp32->bf16; Q,K transposed on PE (d on
    partitions) for the QK^T contraction.
  - S = Q@K^T built per 128-row q-tile in PSUM [128,512] chunks (4 d-step
    matmuls), plus one extra matmul lhsT=(-9984*I128), rhs=mask_bf16 that adds
    -9984 to masked logits in-place in PSUM (exp then underflows to 0).
  - ACT exp reads PSUM, applies the 1/sqrt(512) softmax scale via the free
    affine `scale`, writes bf16 P and emits per-partition row-sum partials via
    accum_out (fp32) -- softmax denominators for free.
  - DVE: sum partials, reciprocal, normalize P (bf16 4x mode); normalized P is
    DMA'd to HBM with SWDGE cast bf16->fp32 as the attn output.
  - P (unnormalized) is PE-transposed per 128x128 tile into PSUM, copied to
    SBUF, used as lhsT for the P@V matmul; PV output rows are scaled by the
    reciprocal row-sum on ACT (copy w/ scale AP) and stored.
"""

import math

import numpy as np

B, S, D = 16, 2048, 512
N_CORES = 8
BPC = B // N_CORES  # batches per core
SOFTMAX_SCALE = 1.0 / math.sqrt(512.0)
MASK_NEG = -9984.0  # exactly representable in bf16; -9984/sqrt(512) ~ -441

P = 128  # partitions


def build_nc(bpc=BPC, s=S, d=D, loop_n=1):
    import concourse.mybir as mybir
    import concourse.tile as tile
    from concourse import bacc

    fp32 = mybir.dt.float32
    bf16 = mybir.dt.bfloat16
    u8 = mybir.dt.uint8

    st = s // P      # number of 128-row s-tiles (16)
    dt_n = d // P    # number of 128-row d-tiles (4)
    kc_n = s // 512  # number of 512-wide k chunks (4)

    nc = bacc.Bacc("TRN2", target_bir_lowering=False, debug=False)

    q_dram = nc.dram_tensor("q", [bpc, s, d], fp32, kind="ExternalInput").ap()
    k_dram = nc.dram_tensor("k", [bpc, s, d], fp32, kind="ExternalInput").ap()
    v_dram = nc.dram_tensor("v", [bpc, s, d], fp32, kind="ExternalInput").ap()
    m_dram = nc.dram_tensor("mask", [bpc, s, s], u8, kind="ExternalInput").ap()
    out_dram = nc.dram_tensor("out", [bpc, s, d], fp32, kind="ExternalOutput").ap()
    attn_dram = nc.dram_tensor("attn", [bpc, s, s], fp32, kind="ExternalOutput").ap()

    with tile.TileContext(nc) as tc:
        with (
            tc.tile_pool(name="singles", bufs=1) as singles,
            tc.tile_pool(name="big", bufs=2) as big,
            tc.tile_pool(name="work", bufs=4) as work,
            tc.tile_pool(name="pt", bufs=3) as ptpool,
            tc.tile_pool(name="small", bufs=4) as small,
            tc.tile_pool(name="ps_s", bufs=4, space="PSUM") as ps_s,
            tc.tile_pool(name="ps_t", bufs=2, space="PSUM") as ps_t,
            tc.tile_pool(name="ps_o", bufs=2, space="PSUM") as ps_o,
        ):
            # constants: identity (transpose helper) and -9984*I (mask add)
            ident = singles.tile([P, P], bf16)
            nc.gpsimd.memset(ident, 0.0)
            nc.gpsimd.affine_select(
                out=ident, in_=ident,
                compare_op=mybir.AluOpType.not_equal, fill=1.0,
                base=0, pattern=[[-1, P]], channel_multiplier=1,
            )
            neg_ident = singles.tile([P, P], bf16)
            nc.gpsimd.memset(neg_ident, 0.0)
            nc.gpsimd.affine_select(
                out=neg_ident, in_=neg_ident,
                compare_op=mybir.AluOpType.not_equal, fill=MASK_NEG,
                base=0, pattern=[[-1, P]], channel_multiplier=1,
            )

            def load_transposed(src_dram_b, name):
                """Load [s, d] fp32 DRAM -> SBUF bf16 [P, dt_n, s] with d on
                partitions (via natural load + PE transpose). PSUM->SBUF
                copies alternate DVE/ACT to halve the preload copy latency."""
                nat = big.tile([P, st, d], bf16, name="nat", tag="nat")
                nc.gpsimd.dma_start(
                    out=nat, in_=src_dram_b.rearrange("(t p) d -> p t d", p=P)
                )
                tsb = big.tile([P, dt_n, s], bf16, name=name)
                ncopy = 0
                for dti in range(dt_n):
                    for sti in range(st):
                        if sti % 4 == 0:
                            ptp = ps_t.tile([P, 4, P], bf16, name="ptp", tag="ptp")
                        nc.tensor.transpose(
                            ptp[:, sti % 4, :],
                            nat[:, sti, dti * P:(dti + 1) * P],
                            ident,
                        )
                        if sti % 4 == 3:
                            nc.vector.tensor_copy(
                                out=tsb[:, dti, (sti - 3) * P:(sti + 1) * P],
                                in_=ptp.rearrange("p a b -> p (a b)"),
                            )
                            ncopy += 1
                return tsb

            def transpose_block(prev, kc):
                """Emit transposes [4*kc .. 4*kc+3] of the previous q-tile's P
                plus the PSUM->SBUF copy of the block (DVE)."""
                p_bf, pt_sb = prev["p_bf"], prev["pt_sb"]
                ptp = ps_t.tile([P, 4, P], bf16, name="ptp", tag="ptp")
                for j in range(4):
                    kt = 4 * kc + j
                    nc.tensor.transpose(
                        ptp[:, j, :], p_bf[:, kt * P:(kt + 1) * P], ident
                    )
                nc.vector.tensor_copy(
                    out=pt_sb[:, 4 * kc:4 * kc + 4, :].rearrange("p a b -> p (a b)"),
                    in_=ptp.rearrange("p a b -> p (a b)"),
                )

            def pv_block(prev):
                """P@V for the previous q-tile + row rescale + out store."""
                b, qt = prev["b"], prev["qt"]
                qsl = slice(qt * P, (qt + 1) * P)
                ps_out = ps_o.tile([P, d], fp32, name="ps_out", tag="ps_out")
                for kt in range(st):
                    nc.tensor.matmul(
                        ps_out,
                        lhsT=prev["pt_sb"][:, kt, :],
                        rhs=prev["v_sb"][:, kt, :],
                        start=(kt == 0),
                        stop=(kt == st - 1),
                    )
                out_sb = work.tile([P, d], fp32, name="out_sb", tag="out_sb")
                nc.scalar.mul(out_sb, ps_out, prev["rinv"])
                nc.sync.dma_start(out=out_dram[b, qsl, :], in_=out_sb)

            def qtile(b, qt, qt_sb, kt_sb, v_sb, prev):
                """QK^T+mask+exp for (b, qt), with the previous q-tile's
                transposes interleaved between QK chunks and its PV after."""
                qsl = slice(qt * P, (qt + 1) * P)
                mask_bf = work.tile([P, s], bf16, name="mask_bf", tag="mask_bf")
                nc.gpsimd.dma_start(out=mask_bf, in_=m_dram[b, qsl, :])

                p_bf = work.tile([P, s], bf16, name="p_bf", tag="p_bf")
                rs_parts = small.tile([P, kc_n], fp32, name="rs_parts")
                for kc in range(kc_n):
                    ksl = slice(kc * 512, (kc + 1) * 512)
                    ps = ps_s.tile([P, 512], fp32, name="ps", tag="ps")
                    for dti in range(dt_n):
                        nc.tensor.matmul(
                            ps,
                            lhsT=qt_sb[:, dti, qsl],
                            rhs=kt_sb[:, dti, ksl],
                            start=(dti == 0),
                            stop=False,
                        )
                    nc.tensor.matmul(
                        ps, lhsT=neg_ident, rhs=mask_bf[:, ksl],
                        start=False, stop=True,
                    )
                    nc.scalar.activation(
                        out=p_bf[:, ksl], in_=ps,
                        func=mybir.ActivationFunctionType.Exp,
                        scale=SOFTMAX_SCALE,
                        accum_out=rs_parts[:, kc:kc + 1],
                    )
                    if prev is not None:
                        transpose_block(prev, kc)
                if prev is not None:
                    pv_block(prev)

                rinv = small.tile([P, 1], fp32, name="rinv", tag="rinv")
                nc.vector.reduce_sum(
                    out=rinv, in_=rs_parts, axis=mybir.AxisListType.X
                )
                nc.vector.reciprocal(out=rinv, in_=rinv)

                p_norm = work.tile([P, s], bf16, name="p_norm", tag="p_norm")
                nc.vector.tensor_scalar_mul(p_norm, p_bf, rinv)
                nc.gpsimd.dma_start(out=attn_dram[b, qsl, :], in_=p_norm)

                pt_sb = ptpool.tile([P, st, P], bf16, name="pt_sb", tag="pt_sb")
                return {
                    "b": b, "qt": qt, "p_bf": p_bf, "rinv": rinv,
                    "v_sb": v_sb, "pt_sb": pt_sb,
                }

            def emit_body():
                prev = None
                for b in range(bpc):
                    qt_sb = load_transposed(q_dram[b], "qt_sb")
                    kt_sb = load_transposed(k_dram[b], "kt_sb")
                    v_sb = big.tile([P, st, d], bf16, name="v_sb", tag="v_sb")
                    nc.gpsimd.dma_start(
                        out=v_sb, in_=v_dram[b].rearrange("(t p) d -> p t d", p=P)
                    )
                    for qt in range(st):
                        prev = qtile(b, qt, qt_sb, kt_sb, v_sb, prev)
                # drain the last q-tile
                for kc in range(kc_n):
                    transpose_block(prev, kc)
                pv_block(prev)

            if loop_n > 1:
                with tc.For_i(0, loop_n, 1):
                    emit_body()
            else:
                emit_body()

    nc.compile()
    return nc


_CACHE = {}


def _get_nc():
    if "nc" not in _CACHE:
        _CACHE["nc"] = build_nc()
    return _CACHE["nc"]


def _get_exec():
    """Compile (once) the 8-core shard_map executable wrapping the Bass NEFF.

    Mirrors concourse.bass2jax.run_bass_via_pjrt's multi-core path, but keeps
    the jitted callable so repeat executions don't re-trace/re-compile and so
    inputs can be pre-placed on device for exec-only timing.
    """
    if "exec" in _CACHE:
        return _CACHE["exec"]

    import jax
    from jax.sharding import Mesh, PartitionSpec
    import concourse.mybir as mybir
    from concourse import bass2jax

    try:
        from jax.experimental.shard_map import shard_map
    except ImportError:  # newer jax
        from jax.sharding import shard_map  # type: ignore

    bass2jax.install_neuronx_cc_hook()
    nc = _get_nc()

    partition_name = (
        nc.partition_id_tensor.name if nc.partition_id_tensor else None
    )
    in_names, out_names, out_avals = [], [], []
    for alloc in nc.m.functions[0].allocations:
        if not isinstance(alloc, mybir.MemoryLocationSet):
            continue
        name = alloc.memorylocations[0].name
        if alloc.kind == "ExternalInput":
            if name != partition_name:
                in_names.append(name)
        elif alloc.kind == "ExternalOutput":
            out_names.append(name)
            out_avals.append(
                jax.core.ShapedArray(
                    tuple(alloc.tensor_shape), mybir.dt.np(alloc.dtype)
                )
            )
    n_params = len(in_names)
    n_outs = len(out_avals)
    all_in_names = list(in_names) + list(out_names)
    if partition_name is not None:
        all_in_names.append(partition_name)
    donate = tuple(range(n_params, n_params + n_outs))

    def _body(*args):
        operands = list(args)
        if partition_name is not None:
            operands.append(bass2jax.partition_id_tensor())
        outs = bass2jax._bass_exec_p.bind(
            *operands,
            out_avals=tuple(out_avals),
            in_names=tuple(all_in_names),
            out_names=tuple(out_names),
            lowering_input_output_aliases=(),
            sim_require_finite=True,
            sim_require_nnan=True,
            nc=nc,
        )
        return tuple(outs)

    devices = jax.devices()[:N_CORES]
    mesh = Mesh(np.asarray(devices), ("core",))
    in_specs = (PartitionSpec("core"),) * (n_params + n_outs)
    out_specs = (PartitionSpec("core"),) * n_outs
    sharded = jax.jit(
        shard_map(
            _body, mesh=mesh, in_specs=in_specs, out_specs=out_specs,
            check_rep=False,
        ),
        donate_argnums=donate,
        keep_unused=True,
    )
    _CACHE["exec"] = (sharded, mesh, in_names, out_names, out_avals)
    return _CACHE["exec"]


def _prep_inputs(q, k, v, mask):
    q = np.ascontiguousarray(np.asarray(q, dtype=np.float32))
    k = np.ascontiguousarray(np.asarray(k, dtype=np.float32))
    v = np.ascontiguousarray(np.asarray(v, dtype=np.float32))
    mask_u8 = np.ascontiguousarray(np.asarray(mask)).astype(np.uint8)
    return {"q": q, "k": k, "v": v, "mask": mask_u8}


def _device_args(arrs):
    """Place global inputs + fresh zero output buffers on the mesh."""
    import jax
    from jax.sharding import NamedSharding, PartitionSpec

    sharded, mesh, in_names, out_names, out_avals = _get_exec()
    sh = NamedSharding(mesh, PartitionSpec("core"))
    args = [jax.device_put(arrs[n], sh) for n in in_names]
    for av in out_avals:
        z = np.zeros((N_CORES * av.shape[0], *av.shape[1:]), av.dtype)
        args.append(jax.device_put(z, sh))
    return args


def run(q, k, v, mask):
    from concourse._compat import axon_active

    arrs = _prep_inputs(q, k, v, mask)
    if not axon_active():
        # native NRT path (no axon tunnel): SPMD with per-core input maps
        from concourse.bass_utils import run_bass_kernel_spmd

        nc = _get_nc()
        in_maps = []
        for c in range(N_CORES):
            sl = slice(c * BPC, (c + 1) * BPC)
            in_maps.append(
                {n: np.ascontiguousarray(arrs[n][sl]) for n in
                 ("q", "k", "v", "mask")}
            )
        res = run_bass_kernel_spmd(nc, in_maps, core_ids=list(range(N_CORES)))
        out = np.concatenate([r["out"] for r in res.results], axis=0)
        attn = np.concatenate([r["attn"] for r in res.results], axis=0)
        return out, attn

    import jax

    sharded, mesh, in_names, out_names, out_avals = _get_exec()
    args = _device_args(arrs)
    outs = jax.block_until_ready(sharded(*args))
    by_name = {n: np.asarray(outs[i]) for i, n in enumerate(out_names)}
    return by_name["out"], by_name["attn"]


def timed_run(q, k, v, mask, iters=3):
    """Returns (outputs, per-iteration wall seconds for exec-only calls)."""
    import time

    import jax

    arrs = _prep_inputs(q, k, v, mask)
    sharded, mesh, in_names, out_names, out_avals = _get_exec()

    args = _device_args(arrs)
    outs = jax.block_until_ready(sharded(*args))  # warm-up / compile
    times = []
    for _ in range(iters):
        args = _device_args(arrs)
        jax.block_until_ready(args)
        t0 = time.perf_counter()
        o = jax.block_until_ready(sharded(*args))
        times.append(time.perf_counter() - t0)
        del o
    by_name = {n: np.asarray(outs[i]) for i, n in enumerate(out_names)}
    return (by_name["out"], by_name["attn"]), times


def kernel(q, k, v, mask):
    out, attn = run(q, k, v, mask)
    return out, attn
